# revision 10
# baseline (speedup 1.0000x reference)
"""Trainium2 Bass kernel for nn_EnergyDistributionCNN (3x3 conv -> unfold ->
softmax over patch -> weighted -> fold overlap-add), 8 NeuronCores.

Math (algebraically identical to the torch/jax reference):
    out = conv3x3(x, k)            cross-correlation, zero pad 1
    E   = exp(out)
    Z   = boxsum3x3(E padded with ONES)   (zero pads contribute exp(0)=1)
    U   = x / Z
    S   = boxsum3x3(U zero-padded)
    result = E * S

Sharding: row-block across 8 cores with a 3-row halo sliced on the host
(zero-filled at the global edges) -- no device-to-device communication.
Global boundary rows are handled uniformly by a per-row mask fused into the
exp's per-partition scale (exp(0*out)=1); boundary columns by host zero
padding plus static edge memsets.

On-core layout: rows on partitions, cols on the free dim, processed in
row-tiles (<=122 output rows) x width-halves. All vertical stencil mixing
runs on the TensorEngine via banded matrices; horizontal mixing is 3
column-shifted matmuls accumulated in PSUM. Everything on the PE uses
fp32r (full-rate moving operand, ~11-bit mantissa); the conv -- whose
error exp() amplifies -- is error-compensated with a hi/lo split:
    conv = Mhi @ Xhi + Mhi @ Xlo + Mlo @ Xhi       (~fp32 quality)
where Xhi is the fp32r-rounded x (DVE copy) and Xlo = x - Xhi.
exp runs on the ScalarEngine directly from conv's PSUM; 1/Z uses the DVE
fast reciprocal (~18 bits). Band row-mappings put every compute op at
partition base 0; the valid output rows sit at partitions [2, R+2), which
the (partition-unrestricted) output DMA reads.
"""

from contextlib import ExitStack

import numpy as np

import concourse.bacc as bacc
import concourse.mybir as mybir
import concourse.tile as tile
from concourse._compat import with_exitstack
from concourse.bass_utils import run_bass_kernel_spmd

F32 = mybir.dt.float32
F32R = mybir.dt.float32r

H = 4096
W = 4096
N_CORES = 8
RC = H // N_CORES  # rows per core
HALO = 3
RT = 122   # output rows per row-tile (RT + 6 <= 128 partitions)
WS = 2     # width splits (SBUF capacity)
WH = W // WS
C = 512    # matmul column chunk = one fp32 PSUM bank
NBUFS = 3
PS_BUFS = 3


# ---------------------------------------------------------------- host side

def _make_bands(k: np.ndarray) -> np.ndarray:
    """bands[v][p, m] = k[p-m, v] (conv, v=0..2); bands[3] = BB ones with
    p-m in 0..2 (S matmul); bands[4] = BT ones with m-p in 0..2 (Z).
    bands[5..9]: same five patterns as 4x block-diagonal 32x32 blocks, for
    the column-folded last row-tile."""
    bands = np.zeros((10, 128, 128), np.float32)
    idx = np.arange(128)
    for d in range(3):
        p = idx[d:]
        m = idx[: 128 - d]
        for v in range(3):
            bands[v, p, m] = k[d, v]
        bands[3, p, m] = 1.0
        bands[4, m, p] = 1.0
    for i in range(5):
        blk = bands[i][:32, :32]
        for b in range(4):
            bands[5 + i][32 * b : 32 * b + 32, 32 * b : 32 * b + 32] = blk
    return bands


def _make_core_inputs(x: np.ndarray, bands: np.ndarray, core: int):
    r0 = core * RC
    lo, hi = r0 - HALO, r0 + RC + HALO
    # 26 extra zero rows let the folded last tile load full 32-row blocks
    xh = np.zeros((RC + 2 * HALO + 26, W + 2 * HALO), np.float32)
    s_lo, s_hi = max(lo, 0), min(hi, H)
    xh[s_lo - lo : s_hi - lo, HALO : HALO + W] = x[s_lo:s_hi]
    gl = np.arange(lo, hi)
    mask = ((gl >= 0) & (gl < H)).astype(np.float32)[:, None]
    return {"xh": xh, "mask": mask, "bands": bands}


def _make_tiles():
    tiles = []
    o = 0
    while o < RC:
        R = min(RT, RC - o)
        tiles.append((o, R))
        o += R
    return tiles


def _chunks(total: int):
    out = []
    s = 0
    while s < total:
        out.append((s, min(C, total - s)))
        s += C
    return out


# -------------------------------------------------------------- device side

@with_exitstack
def _energy_body(ctx: ExitStack, tc, out_d, xh_d, mask_d, bands_d):
    nc = tc.nc
    Exp = mybir.ActivationFunctionType.Exp

    # ---- constants: band matrices, hi/lo split on device ----
    consts = ctx.enter_context(tc.tile_pool(name="consts", bufs=1))

    def load_bands(base, suffix):
        mhi, mlo = [], []
        for v in range(3):
            mf = consts.tile([128, 128], F32, name=f"mf{suffix}{v}")
            nc.sync.dma_start(out=mf, in_=bands_d[base + v])
            hi = consts.tile([128, 128], F32R, name=f"mhi{suffix}{v}")
            nc.vector.tensor_copy(out=hi, in_=mf)
            mhi.append(hi)
            lo = consts.tile([128, 128], F32R, name=f"mlo{suffix}{v}")
            nc.vector.tensor_sub(out=lo, in0=mf, in1=hi)
            mlo.append(lo)
        bb = consts.tile([128, 128], F32R, name=f"bb{suffix}")
        nc.gpsimd.dma_start(out=bb, in_=bands_d[base + 3])
        bt = consts.tile([128, 128], F32R, name=f"bt{suffix}")
        nc.gpsimd.dma_start(out=bt, in_=bands_d[base + 4])
        return mhi, mlo, bb, bt

    Mhi, Mlo, BB, BT = load_bands(0, "")
    MhiF, MloF, BBF, BTF = load_bands(5, "f")
    SEGW = WH // 4

    xpool = ctx.enter_context(tc.tile_pool(name="xp", bufs=NBUFS))
    xhip = ctx.enter_context(tc.tile_pool(name="xhip", bufs=NBUFS))
    xlop = ctx.enter_context(tc.tile_pool(name="xlop", bufs=NBUFS))
    epool = ctx.enter_context(tc.tile_pool(name="ep", bufs=NBUFS))
    upool = ctx.enter_context(tc.tile_pool(name="up", bufs=NBUFS))
    rzpool = ctx.enter_context(tc.tile_pool(name="rzp", bufs=3))
    respool = ctx.enter_context(tc.tile_pool(name="resp", bufs=NBUFS))
    mpool = ctx.enter_context(tc.tile_pool(name="mp", bufs=2))
    ps_conv = ctx.enter_context(tc.tile_pool(name="psc", bufs=PS_BUFS, space="PSUM"))
    ps_z = ctx.enter_context(tc.tile_pool(name="psz", bufs=2, space="PSUM"))
    ps_s = ctx.enter_context(tc.tile_pool(name="pss", bufs=2, space="PSUM"))

    tiles = _make_tiles()
    for o, R in tiles:
        folded = (o, R) == tiles[-1] and len(tiles) > 1 and R <= 26
        mk = mpool.tile([128, 1], F32, tag="mk")
        if folded:
            nc.vector.memset(mk, 0.0)
            for b in range(4):
                nc.sync.dma_start(
                    out=mk[32 * b : 32 * b + R + 4], in_=mask_d[o + 1 : o + R + 5, :]
                )
        else:
            nc.sync.dma_start(out=mk[: R + 4], in_=mask_d[o + 1 : o + R + 5, :])

        if folded:
            # Column-folded last tile: 4 width-segments of each half stacked
            # on 32-partition blocks, block-diagonal bands, ops span all 128
            # partitions (off-band lanes hold finite junk; masked exp gives
            # E=1 and the extended Z band keeps Z>0 there).
            for h in range(WS):
                g0 = h * WH
                X = xpool.tile([128, WH + 6], F32, tag="X")
                for b in range(4):
                    nc.sync.dma_start(
                        out=X[32 * b : 32 * b + 32, : SEGW + 6],
                        in_=xh_d[o : o + 32, g0 + b * SEGW : g0 + b * SEGW + SEGW + 6],
                    )
                Xhi = xhip.tile([128, WH + 6], F32R, tag="Xhi")
                nc.vector.tensor_copy(out=Xhi[:, : SEGW + 6], in_=X[:, : SEGW + 6])
                Xlo = xlop.tile([128, WH + 6], F32R, tag="Xlo")
                nc.vector.tensor_sub(
                    out=Xlo[:, : SEGW + 6],
                    in0=X[:, : SEGW + 6],
                    in1=Xhi[:, : SEGW + 6],
                )

                E = epool.tile([128, WH + 4], F32R, tag="E")
                for cs, cl in _chunks(SEGW + 4):
                    pc = ps_conv.tile([128, C], F32, tag="pc")
                    mms = []
                    for v in range(3):
                        mms.append((MhiF[v], Xhi, v))
                        mms.append((MhiF[v], Xlo, v))
                        mms.append((MloF[v], Xhi, v))
                    for i, (mband, xop, v) in enumerate(mms):
                        nc.tensor.matmul(
                            pc[:, :cl],
                            mband,
                            xop[:, cs + v : cs + v + cl],
                            start=(i == 0),
                            stop=(i == len(mms) - 1),
                        )
                    nc.scalar.activation(E[:, cs : cs + cl], pc[:, :cl], Exp, scale=mk)
                if h == 0:
                    nc.vector.memset(E[0:32, 0:2].bitcast(F32), 1.0)
                if h == WS - 1:
                    nc.vector.memset(E[96:128, SEGW + 2 : SEGW + 4].bitcast(F32), 1.0)

                U = upool.tile([128, WH + 2], F32R, tag="U")
                for cs, cl in _chunks(SEGW + 2):
                    pz = ps_z.tile([128, C], F32, tag="pz")
                    for v in range(3):
                        nc.tensor.matmul(
                            pz[:, :cl],
                            BTF,
                            E[:, cs + v : cs + v + cl],
                            start=(v == 0),
                            stop=(v == 2),
                        )
                    Rz = rzpool.tile([128, C], F32, tag="Rz")
                    nc.vector.reciprocal_approx_fast(out=Rz[:, :cl], in_=pz[:, :cl])
                    nc.vector.tensor_mul(
                        out=U[:, cs : cs + cl],
                        in0=X[:, cs + 2 : cs + 2 + cl],
                        in1=Rz[:, :cl],
                    )
                if h == 0:
                    nc.vector.memset(U[0:32, 0:1].bitcast(F32), 0.0)
                if h == WS - 1:
                    nc.vector.memset(U[96:128, SEGW + 1 : SEGW + 2].bitcast(F32), 0.0)

                res = respool.tile([128, WH], F32, tag="res")
                for cs, cl in _chunks(SEGW):
                    ps = ps_s.tile([128, C], F32, tag="ps")
                    for v in range(3):
                        nc.tensor.matmul(
                            ps[:, :cl],
                            BBF,
                            U[:, cs + v : cs + v + cl],
                            start=(v == 0),
                            stop=(v == 2),
                        )
                    nc.vector.tensor_mul(
                        out=res[:, cs : cs + cl],
                        in0=E[:, cs + 2 : cs + 2 + cl],
                        in1=ps[:, :cl],
                    )
                for b in range(4):
                    nc.sync.dma_start(
                        out=out_d[o : o + R, g0 + b * SEGW : g0 + (b + 1) * SEGW],
                        in_=res[32 * b + 2 : 32 * b + 2 + R, :SEGW],
                    )
            continue

        for h in range(WS):
            g0 = h * WH
            # X[p, j] <-> (row r-3+p, global col g0-3+j)
            X = xpool.tile([128, WH + 6], F32, tag="X")
            nc.sync.dma_start(
                out=X[: R + 6, :], in_=xh_d[o : o + R + 6, g0 : g0 + WH + 6]
            )
            Xhi = xhip.tile([128, WH + 6], F32R, tag="Xhi")
            nc.vector.tensor_copy(out=Xhi[: R + 6, :], in_=X[: R + 6, :])
            Xlo = xlop.tile([128, WH + 6], F32R, tag="Xlo")
            nc.vector.tensor_sub(
                out=Xlo[: R + 6, :], in0=X[: R + 6, :], in1=Xhi[: R + 6, :]
            )

            # conv + exp -> E[m, e] <-> (row r-2+m, global col g0-2+e)
            E = epool.tile([128, WH + 4], F32R, tag="E")
            for cs, cl in _chunks(WH + 4):
                pc = ps_conv.tile([128, C], F32, tag="pc")
                mms = []
                for v in range(3):
                    mms.append((Mhi[v], Xhi, v))
                    mms.append((Mhi[v], Xlo, v))
                    mms.append((Mlo[v], Xhi, v))
                for i, (mband, xop, v) in enumerate(mms):
                    nc.tensor.matmul(
                        pc[: R + 4, :cl],
                        mband[: R + 6, : R + 4],
                        xop[: R + 6, cs + v : cs + v + cl],
                        start=(i == 0),
                        stop=(i == len(mms) - 1),
                    )
                nc.scalar.activation(
                    E[: R + 4, cs : cs + cl],
                    pc[: R + 4, :cl],
                    Exp,
                    scale=mk[: R + 4],
                )
            # global-edge columns of E represent pad pixels: exp(0) = 1
            if h == 0:
                nc.vector.memset(E[: R + 4, 0:2].bitcast(F32), 1.0)
            if h == WS - 1:
                nc.vector.memset(E[: R + 4, WH + 2 : WH + 4].bitcast(F32), 1.0)

            # Z (vertical via BT, X frame) -> Rz -> U[m, z] (global col g0-1+z)
            U = upool.tile([128, WH + 2], F32R, tag="U")
            for cs, cl in _chunks(WH + 2):
                pz = ps_z.tile([128, C], F32, tag="pz")
                for v in range(3):
                    nc.tensor.matmul(
                        pz[: R + 4, :cl],
                        BT[: R + 4, : R + 4],
                        E[: R + 4, cs + v : cs + v + cl],
                        start=(v == 0),
                        stop=(v == 2),
                    )
                Rz = rzpool.tile([128, C], F32, tag="Rz")
                nc.vector.reciprocal_approx_fast(
                    out=Rz[: R + 4, :cl], in_=pz[: R + 4, :cl]
                )
                nc.vector.tensor_mul(
                    out=U[: R + 4, cs : cs + cl],
                    in0=X[: R + 4, cs + 2 : cs + 2 + cl],
                    in1=Rz[: R + 4, :cl],
                )
            # U at global-edge pad columns is 0 (fold drops OOB)
            if h == 0:
                nc.vector.memset(U[: R + 4, 0:1].bitcast(F32), 0.0)
            if h == WS - 1:
                nc.vector.memset(U[: R + 4, WH + 1 : WH + 2].bitcast(F32), 0.0)

            # S (vertical via BB, E frame) + res = E * S
            res = respool.tile([128, WH], F32, tag="res")
            for cs, cl in _chunks(WH):
                ps = ps_s.tile([128, C], F32, tag="ps")
                for v in range(3):
                    nc.tensor.matmul(
                        ps[: R + 2, :cl],
                        BB[: R + 4, : R + 2],
                        U[: R + 4, cs + v : cs + v + cl],
                        start=(v == 0),
                        stop=(v == 2),
                    )
                nc.vector.tensor_mul(
                    out=res[: R + 2, cs : cs + cl],
                    in0=E[: R + 2, cs + 2 : cs + 2 + cl],
                    in1=ps[: R + 2, :cl],
                )
            # valid output rows sit at partitions [2, R+2)
            nc.sync.dma_start(
                out=out_d[o : o + R, g0 : g0 + WH], in_=res[2 : R + 2, :WH]
            )


_CACHE: dict = {}


def _build():
    if "nc" in _CACHE:
        return _CACHE["nc"]
    nc = bacc.Bacc(
        "TRN2", target_bir_lowering=False, debug=False, num_devices=N_CORES
    )
    xh_d = nc.dram_tensor(
        "xh", (RC + 2 * HALO + 26, W + 2 * HALO), F32, kind="ExternalInput"
    ).ap()
    mask_d = nc.dram_tensor("mask", (RC + 2 * HALO, 1), F32, kind="ExternalInput").ap()
    bands_d = nc.dram_tensor("bands", (10, 128, 128), F32, kind="ExternalInput").ap()
    out_d = nc.dram_tensor("out", (RC, W), F32, kind="ExternalOutput").ap()
    with tile.TileContext(nc) as tc:
        _energy_body(tc, out_d, xh_d, mask_d, bands_d)
    nc.compile()
    _CACHE["nc"] = nc
    return nc


def kernel(shareable_energy: np.ndarray, kernel: np.ndarray, **_run_kw) -> np.ndarray:
    x = np.ascontiguousarray(np.asarray(shareable_energy, np.float32))
    k = np.asarray(kernel, np.float32)
    assert x.shape == (H, W), x.shape
    nc = _build()
    bands = _make_bands(k)
    in_maps = [_make_core_inputs(x, bands, core) for core in range(N_CORES)]
    r = run_bass_kernel_spmd(nc, in_maps, core_ids=list(range(N_CORES)), **_run_kw)
    out = np.concatenate([res["out"] for res in r.results], axis=0)
    if _run_kw:
        _CACHE["last_result"] = r
    return out


# revision 12
# speedup vs baseline: 1.0279x; 1.0279x over previous
"""Trainium2 Bass kernel for nn_EnergyDistributionCNN (3x3 conv -> unfold ->
softmax over patch -> weighted -> fold overlap-add), 8 NeuronCores.

Math (algebraically identical to the torch/jax reference):
    out = conv3x3(x, k)            cross-correlation, zero pad 1
    E   = exp(out)
    Z   = boxsum3x3(E padded with ONES)   (zero pads contribute exp(0)=1)
    U   = x / Z
    S   = boxsum3x3(U zero-padded)
    result = E * S

Sharding: row-block across 8 cores with a 3-row halo sliced on the host
(zero-filled at the global edges) -- no device-to-device communication.
Global boundary rows are handled uniformly by a per-row mask fused into the
exp's per-partition scale (exp(0*out)=1); boundary columns by host zero
padding plus static edge memsets.

On-core layout: rows on partitions, cols on the free dim, processed in
row-tiles (<=122 output rows) x width-halves. All vertical stencil mixing
runs on the TensorEngine via banded matrices; horizontal mixing is 3
column-shifted matmuls accumulated in PSUM. Everything on the PE uses
fp32r (full-rate moving operand, ~11-bit mantissa); the conv -- whose
error exp() amplifies -- is error-compensated with a hi/lo split:
    conv = Mhi @ Xhi + Mhi @ Xlo + Mlo @ Xhi       (~fp32 quality)
where Xhi is the fp32r-rounded x (DVE copy) and Xlo = x - Xhi.
exp runs on the ScalarEngine directly from conv's PSUM; 1/Z uses the DVE
fast reciprocal (~18 bits). Band row-mappings put every compute op at
partition base 0; the valid output rows sit at partitions [2, R+2), which
the (partition-unrestricted) output DMA reads.
"""

from contextlib import ExitStack

import numpy as np

import concourse.bacc as bacc
import concourse.mybir as mybir
import concourse.tile as tile
from concourse._compat import with_exitstack
from concourse.bass_utils import run_bass_kernel_spmd

F32 = mybir.dt.float32
F32R = mybir.dt.float32r

H = 4096
W = 4096
N_CORES = 8
RC = H // N_CORES  # rows per core
HALO = 3
RT = 122   # output rows per row-tile (RT + 6 <= 128 partitions)
WS = 2     # width splits (SBUF capacity)
WH = W // WS
C = 512    # matmul column chunk = one fp32 PSUM bank
NBUFS = 3
PS_BUFS = 3


# ---------------------------------------------------------------- host side

def _make_bands(k: np.ndarray) -> np.ndarray:
    """bands[v][p, m] = k[p-m, v] (conv, v=0..2); bands[3] = BB ones with
    p-m in 0..2 (S matmul); bands[4] = BT ones with m-p in 0..2 (Z).
    bands[5..9]: same five patterns as 4x block-diagonal 32x32 blocks, for
    the column-folded last row-tile."""
    bands = np.zeros((10, 128, 128), np.float32)
    idx = np.arange(128)
    for d in range(3):
        p = idx[d:]
        m = idx[: 128 - d]
        for v in range(3):
            bands[v, p, m] = k[d, v]
        bands[3, p, m] = 1.0
        bands[4, m, p] = 1.0
    for i in range(5):
        blk = bands[i][:32, :32]
        for b in range(4):
            bands[5 + i][32 * b : 32 * b + 32, 32 * b : 32 * b + 32] = blk
    return bands


def _make_core_inputs(x: np.ndarray, bands: np.ndarray, core: int):
    r0 = core * RC
    lo, hi = r0 - HALO, r0 + RC + HALO
    # 26 extra zero rows let the folded last tile load full 32-row blocks
    xh = np.zeros((RC + 2 * HALO + 26, W + 2 * HALO), np.float32)
    s_lo, s_hi = max(lo, 0), min(hi, H)
    xh[s_lo - lo : s_hi - lo, HALO : HALO + W] = x[s_lo:s_hi]
    gl = np.arange(lo, hi)
    mask = ((gl >= 0) & (gl < H)).astype(np.float32)[:, None]
    return {"xh": xh, "mask": mask, "bands": bands}


def _make_tiles():
    tiles = []
    o = 0
    while o < RC:
        R = min(RT, RC - o)
        tiles.append((o, R))
        o += R
    return tiles


def _chunks(total: int):
    out = []
    s = 0
    while s < total:
        out.append((s, min(C, total - s)))
        s += C
    return out


# -------------------------------------------------------------- device side

@with_exitstack
def _energy_body(ctx: ExitStack, tc, out_d, xh_d, mask_d, bands_d):
    nc = tc.nc
    Exp = mybir.ActivationFunctionType.Exp

    # ---- constants: band matrices, hi/lo split on device ----
    consts = ctx.enter_context(tc.tile_pool(name="consts", bufs=1))

    def load_bands(base, suffix):
        mhi, mlo = [], []
        for v in range(3):
            mf = consts.tile([128, 128], F32, name=f"mf{suffix}{v}")
            nc.sync.dma_start(out=mf, in_=bands_d[base + v])
            hi = consts.tile([128, 128], F32R, name=f"mhi{suffix}{v}")
            nc.vector.tensor_copy(out=hi, in_=mf)
            mhi.append(hi)
            lo = consts.tile([128, 128], F32R, name=f"mlo{suffix}{v}")
            nc.vector.tensor_sub(out=lo, in0=mf, in1=hi)
            mlo.append(lo)
        bb = consts.tile([128, 128], F32R, name=f"bb{suffix}")
        nc.gpsimd.dma_start(out=bb, in_=bands_d[base + 3])
        bt = consts.tile([128, 128], F32R, name=f"bt{suffix}")
        nc.gpsimd.dma_start(out=bt, in_=bands_d[base + 4])
        return mhi, mlo, bb, bt

    Mhi, Mlo, BB, BT = load_bands(0, "")
    MhiF, MloF, BBF, BTF = load_bands(5, "f")
    SEGW = WH // 4

    xpool = ctx.enter_context(tc.tile_pool(name="xp", bufs=NBUFS))
    xhip = ctx.enter_context(tc.tile_pool(name="xhip", bufs=NBUFS))
    xlop = ctx.enter_context(tc.tile_pool(name="xlop", bufs=NBUFS))
    epool = ctx.enter_context(tc.tile_pool(name="ep", bufs=NBUFS))
    upool = ctx.enter_context(tc.tile_pool(name="up", bufs=NBUFS))
    rzpool = ctx.enter_context(tc.tile_pool(name="rzp", bufs=3))
    respool = ctx.enter_context(tc.tile_pool(name="resp", bufs=NBUFS))
    mpool = ctx.enter_context(tc.tile_pool(name="mp", bufs=2))
    ps_conv = ctx.enter_context(tc.tile_pool(name="psc", bufs=PS_BUFS, space="PSUM"))
    ps_z = ctx.enter_context(tc.tile_pool(name="psz", bufs=2, space="PSUM"))
    ps_s = ctx.enter_context(tc.tile_pool(name="pss", bufs=2, space="PSUM"))

    tiles = _make_tiles()

    def fold_unit(o, R, h):
        # Column-folded last row-tile: 4 width-segments of one half stacked
        # on 32-partition blocks, block-diagonal bands, ops span all 128
        # partitions (off-band lanes hold finite junk; masked exp gives
        # E=1 and the extended Z band keeps Z>0 there).
        mk = mpool.tile([128, 1], F32, tag="mk")
        nc.vector.memset(mk, 0.0)
        for b in range(4):
            nc.sync.dma_start(
                out=mk[32 * b : 32 * b + R + 4], in_=mask_d[o + 1 : o + R + 5, :]
            )
        if True:
            if True:
                g0 = h * WH
                X = xpool.tile([128, WH + 6], F32, tag="X")
                for b in range(4):
                    nc.sync.dma_start(
                        out=X[32 * b : 32 * b + 32, : SEGW + 6],
                        in_=xh_d[o : o + 32, g0 + b * SEGW : g0 + b * SEGW + SEGW + 6],
                    )
                Xhi = xhip.tile([128, WH + 6], F32R, tag="Xhi")
                nc.vector.tensor_copy(out=Xhi[:, : SEGW + 6], in_=X[:, : SEGW + 6])
                Xlo = xlop.tile([128, WH + 6], F32R, tag="Xlo")
                nc.vector.tensor_sub(
                    out=Xlo[:, : SEGW + 6],
                    in0=X[:, : SEGW + 6],
                    in1=Xhi[:, : SEGW + 6],
                )

                E = epool.tile([128, WH + 4], F32R, tag="E")
                for cs, cl in _chunks(SEGW + 4):
                    pc = ps_conv.tile([128, C], F32, tag="pc")
                    mms = []
                    for v in range(3):
                        mms.append((MhiF[v], Xhi, v))
                        mms.append((MhiF[v], Xlo, v))
                        mms.append((MloF[v], Xhi, v))
                    for i, (mband, xop, v) in enumerate(mms):
                        nc.tensor.matmul(
                            pc[:, :cl],
                            mband,
                            xop[:, cs + v : cs + v + cl],
                            start=(i == 0),
                            stop=(i == len(mms) - 1),
                        )
                    nc.scalar.activation(E[:, cs : cs + cl], pc[:, :cl], Exp, scale=mk)
                if h == 0:
                    nc.vector.memset(E[0:32, 0:2].bitcast(F32), 1.0)
                if h == WS - 1:
                    nc.vector.memset(E[96:128, SEGW + 2 : SEGW + 4].bitcast(F32), 1.0)

                U = upool.tile([128, WH + 2], F32R, tag="U")
                for cs, cl in _chunks(SEGW + 2):
                    pz = ps_z.tile([128, C], F32, tag="pz")
                    for v in range(3):
                        nc.tensor.matmul(
                            pz[:, :cl],
                            BTF,
                            E[:, cs + v : cs + v + cl],
                            start=(v == 0),
                            stop=(v == 2),
                        )
                    Rz = rzpool.tile([128, C], F32, tag="Rz")
                    nc.vector.reciprocal_approx_fast(out=Rz[:, :cl], in_=pz[:, :cl])
                    nc.vector.tensor_mul(
                        out=U[:, cs : cs + cl],
                        in0=X[:, cs + 2 : cs + 2 + cl],
                        in1=Rz[:, :cl],
                    )
                if h == 0:
                    nc.vector.memset(U[0:32, 0:1].bitcast(F32), 0.0)
                if h == WS - 1:
                    nc.vector.memset(U[96:128, SEGW + 1 : SEGW + 2].bitcast(F32), 0.0)

                res = respool.tile([128, WH], F32, tag="res")
                for cs, cl in _chunks(SEGW):
                    ps = ps_s.tile([128, C], F32, tag="ps")
                    for v in range(3):
                        nc.tensor.matmul(
                            ps[:, :cl],
                            BBF,
                            U[:, cs + v : cs + v + cl],
                            start=(v == 0),
                            stop=(v == 2),
                        )
                    nc.vector.tensor_mul(
                        out=res[:, cs : cs + cl],
                        in0=E[:, cs + 2 : cs + 2 + cl],
                        in1=ps[:, :cl],
                    )
                for b in range(4):
                    nc.sync.dma_start(
                        out=out_d[o : o + R, g0 + b * SEGW : g0 + (b + 1) * SEGW],
                        in_=res[32 * b + 2 : 32 * b + 2 + R, :SEGW],
                    )
            return

    def normal_tile(o, R):
        mk = mpool.tile([128, 1], F32, tag="mk")
        nc.sync.dma_start(out=mk[: R + 4], in_=mask_d[o + 1 : o + R + 5, :])
        for h in range(WS):
            g0 = h * WH
            # X[p, j] <-> (row r-3+p, global col g0-3+j)
            X = xpool.tile([128, WH + 6], F32, tag="X")
            nc.sync.dma_start(
                out=X[: R + 6, :], in_=xh_d[o : o + R + 6, g0 : g0 + WH + 6]
            )
            Xhi = xhip.tile([128, WH + 6], F32R, tag="Xhi")
            nc.vector.tensor_copy(out=Xhi[: R + 6, :], in_=X[: R + 6, :])
            Xlo = xlop.tile([128, WH + 6], F32R, tag="Xlo")
            nc.vector.tensor_sub(
                out=Xlo[: R + 6, :], in0=X[: R + 6, :], in1=Xhi[: R + 6, :]
            )

            # conv + exp -> E[m, e] <-> (row r-2+m, global col g0-2+e)
            E = epool.tile([128, WH + 4], F32R, tag="E")
            for cs, cl in _chunks(WH + 4):
                pc = ps_conv.tile([128, C], F32, tag="pc")
                mms = []
                for v in range(3):
                    mms.append((Mhi[v], Xhi, v))
                    mms.append((Mhi[v], Xlo, v))
                    mms.append((Mlo[v], Xhi, v))
                for i, (mband, xop, v) in enumerate(mms):
                    nc.tensor.matmul(
                        pc[: R + 4, :cl],
                        mband[: R + 6, : R + 4],
                        xop[: R + 6, cs + v : cs + v + cl],
                        start=(i == 0),
                        stop=(i == len(mms) - 1),
                    )
                nc.scalar.activation(
                    E[: R + 4, cs : cs + cl],
                    pc[: R + 4, :cl],
                    Exp,
                    scale=mk[: R + 4],
                )
            # global-edge columns of E represent pad pixels: exp(0) = 1
            if h == 0:
                nc.vector.memset(E[: R + 4, 0:2].bitcast(F32), 1.0)
            if h == WS - 1:
                nc.vector.memset(E[: R + 4, WH + 2 : WH + 4].bitcast(F32), 1.0)

            # Z (vertical via BT, X frame) -> Rz -> U[m, z] (global col g0-1+z)
            U = upool.tile([128, WH + 2], F32R, tag="U")
            for cs, cl in _chunks(WH + 2):
                pz = ps_z.tile([128, C], F32, tag="pz")
                for v in range(3):
                    nc.tensor.matmul(
                        pz[: R + 4, :cl],
                        BT[: R + 4, : R + 4],
                        E[: R + 4, cs + v : cs + v + cl],
                        start=(v == 0),
                        stop=(v == 2),
                    )
                Rz = rzpool.tile([128, C], F32, tag="Rz")
                nc.vector.reciprocal_approx_fast(
                    out=Rz[: R + 4, :cl], in_=pz[: R + 4, :cl]
                )
                nc.vector.tensor_mul(
                    out=U[: R + 4, cs : cs + cl],
                    in0=X[: R + 4, cs + 2 : cs + 2 + cl],
                    in1=Rz[: R + 4, :cl],
                )
            # U at global-edge pad columns is 0 (fold drops OOB)
            if h == 0:
                nc.vector.memset(U[: R + 4, 0:1].bitcast(F32), 0.0)
            if h == WS - 1:
                nc.vector.memset(U[: R + 4, WH + 1 : WH + 2].bitcast(F32), 0.0)

            # S (vertical via BB, E frame) + res = E * S
            res = respool.tile([128, WH], F32, tag="res")
            for cs, cl in _chunks(WH):
                ps = ps_s.tile([128, C], F32, tag="ps")
                for v in range(3):
                    nc.tensor.matmul(
                        ps[: R + 2, :cl],
                        BB[: R + 4, : R + 2],
                        U[: R + 4, cs + v : cs + v + cl],
                        start=(v == 0),
                        stop=(v == 2),
                    )
                nc.vector.tensor_mul(
                    out=res[: R + 2, cs : cs + cl],
                    in0=E[: R + 2, cs + 2 : cs + 2 + cl],
                    in1=ps[: R + 2, :cl],
                )
            # valid output rows sit at partitions [2, R+2)
            nc.sync.dma_start(
                out=out_d[o : o + R, g0 : g0 + WH], in_=res[2 : R + 2, :WH]
            )

    of, Rf = tiles[-1]
    if len(tiles) > 1 and Rf <= 26:
        # cheap folded units at both pipeline edges: fast fill and drain
        fold_unit(of, Rf, 0)
        for o, R in tiles[:-1]:
            normal_tile(o, R)
        fold_unit(of, Rf, WS - 1)
    else:
        for o, R in tiles:
            normal_tile(o, R)


_CACHE: dict = {}


def _build():
    if "nc" in _CACHE:
        return _CACHE["nc"]
    nc = bacc.Bacc(
        "TRN2", target_bir_lowering=False, debug=False, num_devices=N_CORES
    )
    xh_d = nc.dram_tensor(
        "xh", (RC + 2 * HALO + 26, W + 2 * HALO), F32, kind="ExternalInput"
    ).ap()
    mask_d = nc.dram_tensor("mask", (RC + 2 * HALO, 1), F32, kind="ExternalInput").ap()
    bands_d = nc.dram_tensor("bands", (10, 128, 128), F32, kind="ExternalInput").ap()
    out_d = nc.dram_tensor("out", (RC, W), F32, kind="ExternalOutput").ap()
    with tile.TileContext(nc) as tc:
        _energy_body(tc, out_d, xh_d, mask_d, bands_d)
    nc.compile()
    _CACHE["nc"] = nc
    return nc


def kernel(shareable_energy: np.ndarray, kernel: np.ndarray, **_run_kw) -> np.ndarray:
    x = np.ascontiguousarray(np.asarray(shareable_energy, np.float32))
    k = np.asarray(kernel, np.float32)
    assert x.shape == (H, W), x.shape
    nc = _build()
    bands = _make_bands(k)
    in_maps = [_make_core_inputs(x, bands, core) for core in range(N_CORES)]
    r = run_bass_kernel_spmd(nc, in_maps, core_ids=list(range(N_CORES)), **_run_kw)
    out = np.concatenate([res["out"] for res in r.results], axis=0)
    if _run_kw:
        _CACHE["last_result"] = r
    return out


# revision 13
# speedup vs baseline: 1.0543x; 1.0257x over previous
"""Trainium2 Bass kernel for nn_EnergyDistributionCNN (3x3 conv -> unfold ->
softmax over patch -> weighted -> fold overlap-add), 8 NeuronCores.

Math (algebraically identical to the torch/jax reference):
    out = conv3x3(x, k)            cross-correlation, zero pad 1
    E   = exp(out)
    Z   = boxsum3x3(E padded with ONES)   (zero pads contribute exp(0)=1)
    U   = x / Z
    S   = boxsum3x3(U zero-padded)
    result = E * S

Sharding: row-block across 8 cores with a 3-row halo sliced on the host
(zero-filled at the global edges) -- no device-to-device communication.
Global boundary rows are handled uniformly by a per-row mask fused into the
exp's per-partition scale (exp(0*out)=1); boundary columns by host zero
padding plus static edge memsets.

On-core layout: rows on partitions, cols on the free dim, processed in
row-tiles (<=122 output rows) x width-halves. All vertical stencil mixing
runs on the TensorEngine via banded matrices; horizontal mixing is 3
column-shifted matmuls accumulated in PSUM. Everything on the PE uses
fp32r (full-rate moving operand, ~11-bit mantissa); the conv -- whose
error exp() amplifies -- is error-compensated with a hi/lo split:
    conv = Mhi @ Xhi + Mhi @ Xlo + Mlo @ Xhi       (~fp32 quality)
where Xhi is the fp32r-rounded x (DVE copy) and Xlo = x - Xhi.
exp runs on the ScalarEngine directly from conv's PSUM; 1/Z uses the DVE
fast reciprocal (~18 bits). Band row-mappings put every compute op at
partition base 0; the valid output rows sit at partitions [2, R+2), which
the (partition-unrestricted) output DMA reads.
"""

from contextlib import ExitStack

import numpy as np

import concourse.bacc as bacc
import concourse.mybir as mybir
import concourse.tile as tile
from concourse._compat import with_exitstack
from concourse.bass_utils import run_bass_kernel_spmd

F32 = mybir.dt.float32
F32R = mybir.dt.float32r

H = 4096
W = 4096
N_CORES = 8
RC = H // N_CORES  # rows per core
HALO = 3
RT = 122   # output rows per row-tile (RT + 6 <= 128 partitions)
WS = 2     # width splits (SBUF capacity)
WH = W // WS
C = 512    # matmul column chunk = one fp32 PSUM bank
NBUFS = 3
PS_BUFS = 3


# ---------------------------------------------------------------- host side

def _make_bands(k: np.ndarray) -> np.ndarray:
    """bands[v][p, m] = k[p-m, v] (conv, v=0..2); bands[3] = BB ones with
    p-m in 0..2 (S matmul); bands[4] = BT ones with m-p in 0..2 (Z).
    bands[5..9]: same five patterns as 4x block-diagonal 32x32 blocks, for
    the column-folded last row-tile."""
    bands = np.zeros((10, 128, 128), np.float32)
    idx = np.arange(128)
    for d in range(3):
        p = idx[d:]
        m = idx[: 128 - d]
        for v in range(3):
            bands[v, p, m] = k[d, v]
        bands[3, p, m] = 1.0
        bands[4, m, p] = 1.0
    for i in range(5):
        blk = bands[i][:32, :32]
        for b in range(4):
            bands[5 + i][32 * b : 32 * b + 32, 32 * b : 32 * b + 32] = blk
    return bands


def _make_core_inputs(x: np.ndarray, bands: np.ndarray, core: int):
    r0 = core * RC
    lo, hi = r0 - HALO, r0 + RC + HALO
    # 26 extra zero rows let the folded last tile load full 32-row blocks
    xh = np.zeros((RC + 2 * HALO + 26, W + 2 * HALO), np.float32)
    s_lo, s_hi = max(lo, 0), min(hi, H)
    xh[s_lo - lo : s_hi - lo, HALO : HALO + W] = x[s_lo:s_hi]
    gl = np.arange(lo, hi)
    mask = ((gl >= 0) & (gl < H)).astype(np.float32)[:, None]
    return {"xh": xh, "mask": mask, "bands": bands}


def _make_tiles():
    tiles = []
    o = 0
    while o < RC:
        R = min(RT, RC - o)
        tiles.append((o, R))
        o += R
    return tiles


def _chunks(total: int):
    out = []
    s = 0
    while s < total:
        out.append((s, min(C, total - s)))
        s += C
    return out


# -------------------------------------------------------------- device side

@with_exitstack
def _energy_body(ctx: ExitStack, tc, out_d, xh_d, mask_d, bands_d):
    nc = tc.nc
    Exp = mybir.ActivationFunctionType.Exp

    # ---- constants: ONE DMA for all band matrices, hi/lo split on device;
    # the folded set is materialized first (the first emitted unit needs it)
    consts = ctx.enter_context(tc.tile_pool(name="consts", bufs=1))
    bigb = consts.tile([128, 10 * 128], F32, name="bigb")
    nc.sync.dma_start(
        out=bigb.rearrange("p (i m) -> p i m", i=10),
        in_=bands_d.rearrange("i p m -> p i m"),
    )

    def load_bands(base, suffix):
        mhi, mlo = [], []
        for v in range(3):
            mf = bigb[:, (base + v) * 128 : (base + v + 1) * 128]
            hi = consts.tile([128, 128], F32R, name=f"mhi{suffix}{v}")
            nc.vector.tensor_copy(out=hi, in_=mf)
            mhi.append(hi)
            lo = consts.tile([128, 128], F32R, name=f"mlo{suffix}{v}")
            nc.vector.tensor_sub(out=lo, in0=mf, in1=hi)
            mlo.append(lo)
        bb = consts.tile([128, 128], F32R, name=f"bb{suffix}")
        nc.vector.tensor_copy(out=bb, in_=bigb[:, (base + 3) * 128 : (base + 4) * 128])
        bt = consts.tile([128, 128], F32R, name=f"bt{suffix}")
        nc.vector.tensor_copy(out=bt, in_=bigb[:, (base + 4) * 128 : (base + 5) * 128])
        return mhi, mlo, bb, bt

    MhiF, MloF, BBF, BTF = load_bands(5, "f")
    Mhi, Mlo, BB, BT = load_bands(0, "")
    SEGW = WH // 4

    xpool = ctx.enter_context(tc.tile_pool(name="xp", bufs=NBUFS))
    xhip = ctx.enter_context(tc.tile_pool(name="xhip", bufs=NBUFS))
    xlop = ctx.enter_context(tc.tile_pool(name="xlop", bufs=NBUFS))
    epool = ctx.enter_context(tc.tile_pool(name="ep", bufs=NBUFS))
    upool = ctx.enter_context(tc.tile_pool(name="up", bufs=NBUFS))
    rzpool = ctx.enter_context(tc.tile_pool(name="rzp", bufs=3))
    respool = ctx.enter_context(tc.tile_pool(name="resp", bufs=NBUFS))
    mpool = ctx.enter_context(tc.tile_pool(name="mp", bufs=2))
    ps_conv = ctx.enter_context(tc.tile_pool(name="psc", bufs=PS_BUFS, space="PSUM"))
    ps_z = ctx.enter_context(tc.tile_pool(name="psz", bufs=2, space="PSUM"))
    ps_s = ctx.enter_context(tc.tile_pool(name="pss", bufs=2, space="PSUM"))

    tiles = _make_tiles()

    def fold_unit(o, R, h):
        # Column-folded last row-tile: 4 width-segments of one half stacked
        # on 32-partition blocks, block-diagonal bands, ops span all 128
        # partitions (off-band lanes hold finite junk; masked exp gives
        # E=1 and the extended Z band keeps Z>0 there).
        mk = mpool.tile([128, 1], F32, tag="mk")
        nc.vector.memset(mk, 0.0)
        for b in range(4):
            nc.sync.dma_start(
                out=mk[32 * b : 32 * b + R + 4], in_=mask_d[o + 1 : o + R + 5, :]
            )
        if True:
            if True:
                g0 = h * WH
                X = xpool.tile([128, WH + 6], F32, tag="X")
                for b in range(4):
                    nc.sync.dma_start(
                        out=X[32 * b : 32 * b + 32, : SEGW + 6],
                        in_=xh_d[o : o + 32, g0 + b * SEGW : g0 + b * SEGW + SEGW + 6],
                    )
                Xhi = xhip.tile([128, WH + 6], F32R, tag="Xhi")
                nc.vector.tensor_copy(out=Xhi[:, : SEGW + 6], in_=X[:, : SEGW + 6])
                Xlo = xlop.tile([128, WH + 6], F32R, tag="Xlo")
                nc.vector.tensor_sub(
                    out=Xlo[:, : SEGW + 6],
                    in0=X[:, : SEGW + 6],
                    in1=Xhi[:, : SEGW + 6],
                )

                E = epool.tile([128, WH + 4], F32R, tag="E")
                for cs, cl in _chunks(SEGW + 4):
                    pc = ps_conv.tile([128, C], F32, tag="pc")
                    mms = []
                    for v in range(3):
                        mms.append((MhiF[v], Xhi, v))
                        mms.append((MhiF[v], Xlo, v))
                        mms.append((MloF[v], Xhi, v))
                    for i, (mband, xop, v) in enumerate(mms):
                        nc.tensor.matmul(
                            pc[:, :cl],
                            mband,
                            xop[:, cs + v : cs + v + cl],
                            start=(i == 0),
                            stop=(i == len(mms) - 1),
                        )
                    nc.scalar.activation(E[:, cs : cs + cl], pc[:, :cl], Exp, scale=mk)
                if h == 0:
                    nc.vector.memset(E[0:32, 0:2].bitcast(F32), 1.0)
                if h == WS - 1:
                    nc.vector.memset(E[96:128, SEGW + 2 : SEGW + 4].bitcast(F32), 1.0)

                U = upool.tile([128, WH + 2], F32R, tag="U")
                for cs, cl in _chunks(SEGW + 2):
                    pz = ps_z.tile([128, C], F32, tag="pz")
                    for v in range(3):
                        nc.tensor.matmul(
                            pz[:, :cl],
                            BTF,
                            E[:, cs + v : cs + v + cl],
                            start=(v == 0),
                            stop=(v == 2),
                        )
                    Rz = rzpool.tile([128, C], F32, tag="Rz")
                    nc.vector.reciprocal_approx_fast(out=Rz[:, :cl], in_=pz[:, :cl])
                    nc.vector.tensor_mul(
                        out=U[:, cs : cs + cl],
                        in0=X[:, cs + 2 : cs + 2 + cl],
                        in1=Rz[:, :cl],
                    )
                if h == 0:
                    nc.vector.memset(U[0:32, 0:1].bitcast(F32), 0.0)
                if h == WS - 1:
                    nc.vector.memset(U[96:128, SEGW + 1 : SEGW + 2].bitcast(F32), 0.0)

                res = respool.tile([128, WH], F32, tag="res")
                for cs, cl in _chunks(SEGW):
                    ps = ps_s.tile([128, C], F32, tag="ps")
                    for v in range(3):
                        nc.tensor.matmul(
                            ps[:, :cl],
                            BBF,
                            U[:, cs + v : cs + v + cl],
                            start=(v == 0),
                            stop=(v == 2),
                        )
                    nc.vector.tensor_mul(
                        out=res[:, cs : cs + cl],
                        in0=E[:, cs + 2 : cs + 2 + cl],
                        in1=ps[:, :cl],
                    )
                for b in range(4):
                    nc.sync.dma_start(
                        out=out_d[o : o + R, g0 + b * SEGW : g0 + (b + 1) * SEGW],
                        in_=res[32 * b + 2 : 32 * b + 2 + R, :SEGW],
                    )
            return

    def normal_tile(o, R):
        mk = mpool.tile([128, 1], F32, tag="mk")
        nc.sync.dma_start(out=mk[: R + 4], in_=mask_d[o + 1 : o + R + 5, :])
        for h in range(WS):
            g0 = h * WH
            # X[p, j] <-> (row r-3+p, global col g0-3+j)
            X = xpool.tile([128, WH + 6], F32, tag="X")
            nc.sync.dma_start(
                out=X[: R + 6, :], in_=xh_d[o : o + R + 6, g0 : g0 + WH + 6]
            )
            Xhi = xhip.tile([128, WH + 6], F32R, tag="Xhi")
            nc.vector.tensor_copy(out=Xhi[: R + 6, :], in_=X[: R + 6, :])
            Xlo = xlop.tile([128, WH + 6], F32R, tag="Xlo")
            nc.vector.tensor_sub(
                out=Xlo[: R + 6, :], in0=X[: R + 6, :], in1=Xhi[: R + 6, :]
            )

            # conv + exp -> E[m, e] <-> (row r-2+m, global col g0-2+e)
            E = epool.tile([128, WH + 4], F32R, tag="E")
            for cs, cl in _chunks(WH + 4):
                pc = ps_conv.tile([128, C], F32, tag="pc")
                mms = []
                for v in range(3):
                    mms.append((Mhi[v], Xhi, v))
                    mms.append((Mhi[v], Xlo, v))
                    mms.append((Mlo[v], Xhi, v))
                for i, (mband, xop, v) in enumerate(mms):
                    nc.tensor.matmul(
                        pc[: R + 4, :cl],
                        mband[: R + 6, : R + 4],
                        xop[: R + 6, cs + v : cs + v + cl],
                        start=(i == 0),
                        stop=(i == len(mms) - 1),
                    )
                nc.scalar.activation(
                    E[: R + 4, cs : cs + cl],
                    pc[: R + 4, :cl],
                    Exp,
                    scale=mk[: R + 4],
                )
            # global-edge columns of E represent pad pixels: exp(0) = 1
            if h == 0:
                nc.vector.memset(E[: R + 4, 0:2].bitcast(F32), 1.0)
            if h == WS - 1:
                nc.vector.memset(E[: R + 4, WH + 2 : WH + 4].bitcast(F32), 1.0)

            # Z (vertical via BT, X frame) -> Rz -> U[m, z] (global col g0-1+z)
            U = upool.tile([128, WH + 2], F32R, tag="U")
            for cs, cl in _chunks(WH + 2):
                pz = ps_z.tile([128, C], F32, tag="pz")
                for v in range(3):
                    nc.tensor.matmul(
                        pz[: R + 4, :cl],
                        BT[: R + 4, : R + 4],
                        E[: R + 4, cs + v : cs + v + cl],
                        start=(v == 0),
                        stop=(v == 2),
                    )
                Rz = rzpool.tile([128, C], F32, tag="Rz")
                nc.vector.reciprocal_approx_fast(
                    out=Rz[: R + 4, :cl], in_=pz[: R + 4, :cl]
                )
                nc.vector.tensor_mul(
                    out=U[: R + 4, cs : cs + cl],
                    in0=X[: R + 4, cs + 2 : cs + 2 + cl],
                    in1=Rz[: R + 4, :cl],
                )
            # U at global-edge pad columns is 0 (fold drops OOB)
            if h == 0:
                nc.vector.memset(U[: R + 4, 0:1].bitcast(F32), 0.0)
            if h == WS - 1:
                nc.vector.memset(U[: R + 4, WH + 1 : WH + 2].bitcast(F32), 0.0)

            # S (vertical via BB, E frame) + res = E * S
            res = respool.tile([128, WH], F32, tag="res")
            for cs, cl in _chunks(WH):
                ps = ps_s.tile([128, C], F32, tag="ps")
                for v in range(3):
                    nc.tensor.matmul(
                        ps[: R + 2, :cl],
                        BB[: R + 4, : R + 2],
                        U[: R + 4, cs + v : cs + v + cl],
                        start=(v == 0),
                        stop=(v == 2),
                    )
                nc.vector.tensor_mul(
                    out=res[: R + 2, cs : cs + cl],
                    in0=E[: R + 2, cs + 2 : cs + 2 + cl],
                    in1=ps[: R + 2, :cl],
                )
            # valid output rows sit at partitions [2, R+2)
            nc.sync.dma_start(
                out=out_d[o : o + R, g0 : g0 + WH], in_=res[2 : R + 2, :WH]
            )

    of, Rf = tiles[-1]
    if len(tiles) > 1 and Rf <= 26:
        # cheap folded units at both pipeline edges: fast fill and drain
        fold_unit(of, Rf, 0)
        for o, R in tiles[:-1]:
            normal_tile(o, R)
        fold_unit(of, Rf, WS - 1)
    else:
        for o, R in tiles:
            normal_tile(o, R)


_CACHE: dict = {}


def _build():
    if "nc" in _CACHE:
        return _CACHE["nc"]
    nc = bacc.Bacc(
        "TRN2", target_bir_lowering=False, debug=False, num_devices=N_CORES
    )
    xh_d = nc.dram_tensor(
        "xh", (RC + 2 * HALO + 26, W + 2 * HALO), F32, kind="ExternalInput"
    ).ap()
    mask_d = nc.dram_tensor("mask", (RC + 2 * HALO, 1), F32, kind="ExternalInput").ap()
    bands_d = nc.dram_tensor("bands", (10, 128, 128), F32, kind="ExternalInput").ap()
    out_d = nc.dram_tensor("out", (RC, W), F32, kind="ExternalOutput").ap()
    with tile.TileContext(nc) as tc:
        _energy_body(tc, out_d, xh_d, mask_d, bands_d)
    nc.compile()
    _CACHE["nc"] = nc
    return nc


def kernel(shareable_energy: np.ndarray, kernel: np.ndarray, **_run_kw) -> np.ndarray:
    x = np.ascontiguousarray(np.asarray(shareable_energy, np.float32))
    k = np.asarray(kernel, np.float32)
    assert x.shape == (H, W), x.shape
    nc = _build()
    bands = _make_bands(k)
    in_maps = [_make_core_inputs(x, bands, core) for core in range(N_CORES)]
    r = run_bass_kernel_spmd(nc, in_maps, core_ids=list(range(N_CORES)), **_run_kw)
    out = np.concatenate([res["out"] for res in r.results], axis=0)
    if _run_kw:
        _CACHE["last_result"] = r
    return out


# revision 19
# speedup vs baseline: 1.0611x; 1.0064x over previous
"""Trainium2 Bass kernel for nn_EnergyDistributionCNN (3x3 conv -> unfold ->
softmax over patch -> weighted -> fold overlap-add), 8 NeuronCores.

Math (algebraically identical to the torch/jax reference):
    out = conv3x3(x, k)            cross-correlation, zero pad 1
    E   = exp(out)
    Z   = boxsum3x3(E padded with ONES)   (zero pads contribute exp(0)=1)
    U   = x / Z
    S   = boxsum3x3(U zero-padded)
    result = E * S

Sharding: row-block across 8 cores with a 3-row halo sliced on the host
(zero-filled at the global edges) -- no device-to-device communication.
Global boundary rows are handled uniformly by a per-row mask fused into the
exp's per-partition scale (exp(0*out)=1); boundary columns by host zero
padding plus static edge memsets.

On-core layout: rows on partitions, cols on the free dim, processed in
row-tiles (<=122 output rows) x width-halves. All vertical stencil mixing
runs on the TensorEngine via banded matrices; horizontal mixing is 3
column-shifted matmuls accumulated in PSUM. Everything on the PE uses
fp32r (full-rate moving operand, ~11-bit mantissa); the conv -- whose
error exp() amplifies -- is error-compensated with a hi/lo split:
    conv = Mhi @ Xhi + Mhi @ Xlo + Mlo @ Xhi       (~fp32 quality)
where Xhi is the fp32r-rounded x (DVE copy) and Xlo = x - Xhi.
exp runs on the ScalarEngine directly from conv's PSUM; 1/Z uses the DVE
fast reciprocal (~18 bits). Band row-mappings put every compute op at
partition base 0; the valid output rows sit at partitions [2, R+2), which
the (partition-unrestricted) output DMA reads.
"""

from contextlib import ExitStack

import numpy as np

import concourse.bacc as bacc
import concourse.mybir as mybir
import concourse.tile as tile
from concourse._compat import with_exitstack
from concourse.bass_utils import run_bass_kernel_spmd

F32 = mybir.dt.float32
F32R = mybir.dt.float32r

H = 4096
W = 4096
N_CORES = 8
RC = H // N_CORES  # rows per core
HALO = 3
RT = 122   # output rows per row-tile (RT + 6 <= 128 partitions)
WS = 2     # width splits (SBUF capacity)
WH = W // WS
C = 512    # matmul column chunk = one fp32 PSUM bank
NBUFS = 3
PS_BUFS = 3


# ---------------------------------------------------------------- host side

def _make_bands(k: np.ndarray) -> np.ndarray:
    """bands[v][p, m] = k[p-m, v] (conv, v=0..2); bands[3] = BB ones with
    p-m in 0..2 (S matmul); bands[4] = BT ones with m-p in 0..2 (Z).
    bands[5..9]: same five patterns as 4x block-diagonal 32x32 blocks, for
    the column-folded last row-tile."""
    bands = np.zeros((10, 128, 128), np.float32)
    idx = np.arange(128)
    for d in range(3):
        p = idx[d:]
        m = idx[: 128 - d]
        for v in range(3):
            bands[v, p, m] = k[d, v]
        bands[3, p, m] = 1.0
        bands[4, m, p] = 1.0
    for i in range(5):
        blk = bands[i][:32, :32]
        for b in range(4):
            bands[5 + i][32 * b : 32 * b + 32, 32 * b : 32 * b + 32] = blk
    return bands


def _make_core_inputs(x: np.ndarray, bands: np.ndarray, core: int):
    r0 = core * RC
    lo, hi = r0 - HALO, r0 + RC + HALO
    # 26 extra zero rows let the folded last tile load full 32-row blocks
    xh = np.zeros((RC + 2 * HALO + 26, W + 2 * HALO), np.float32)
    s_lo, s_hi = max(lo, 0), min(hi, H)
    xh[s_lo - lo : s_hi - lo, HALO : HALO + W] = x[s_lo:s_hi]
    gl = np.arange(lo, hi)
    mask = ((gl >= 0) & (gl < H)).astype(np.float32)[:, None]
    return {"xh": xh, "mask": mask, "bands": bands}


def _make_tiles():
    tiles = []
    o = 0
    while o < RC:
        R = min(RT, RC - o)
        tiles.append((o, R))
        o += R
    return tiles


def _chunks(total: int):
    out = []
    s = 0
    while s < total:
        out.append((s, min(C, total - s)))
        s += C
    return out


# -------------------------------------------------------------- device side

@with_exitstack
def _energy_body(ctx: ExitStack, tc, out_d, xh_d, mask_d, bands_d):
    nc = tc.nc
    Exp = mybir.ActivationFunctionType.Exp

    # ---- constants: ONE DMA for all band matrices, hi/lo split on device;
    # the folded set is materialized first (the first emitted unit needs it)
    consts = ctx.enter_context(tc.tile_pool(name="consts", bufs=1))
    bigb = consts.tile([128, 10 * 128], F32, name="bigb")
    nc.sync.dma_start(
        out=bigb.rearrange("p (i m) -> p i m", i=10),
        in_=bands_d.rearrange("i p m -> p i m"),
    )

    def load_bands(base, suffix):
        mhi, mlo = [], []
        for v in range(3):
            mf = bigb[:, (base + v) * 128 : (base + v + 1) * 128]
            hi = consts.tile([128, 128], F32R, name=f"mhi{suffix}{v}")
            nc.vector.tensor_copy(out=hi, in_=mf)
            mhi.append(hi)
            lo = consts.tile([128, 128], F32R, name=f"mlo{suffix}{v}")
            nc.vector.tensor_sub(out=lo, in0=mf, in1=hi)
            mlo.append(lo)
        bb = consts.tile([128, 128], F32R, name=f"bb{suffix}")
        nc.vector.tensor_copy(out=bb, in_=bigb[:, (base + 3) * 128 : (base + 4) * 128])
        bt = consts.tile([128, 128], F32R, name=f"bt{suffix}")
        nc.vector.tensor_copy(out=bt, in_=bigb[:, (base + 4) * 128 : (base + 5) * 128])
        return mhi, mlo, bb, bt

    MhiF, MloF, BBF, BTF = load_bands(5, "f")
    Mhi, Mlo, BB, BT = load_bands(0, "")
    SEGW = WH // 4

    xpool = ctx.enter_context(tc.tile_pool(name="xp", bufs=NBUFS))
    xhip = ctx.enter_context(tc.tile_pool(name="xhip", bufs=NBUFS))
    xlop = ctx.enter_context(tc.tile_pool(name="xlop", bufs=NBUFS))
    epool = ctx.enter_context(tc.tile_pool(name="ep", bufs=NBUFS))
    upool = ctx.enter_context(tc.tile_pool(name="up", bufs=NBUFS))
    rzpool = ctx.enter_context(tc.tile_pool(name="rzp", bufs=3))
    respool = ctx.enter_context(tc.tile_pool(name="resp", bufs=NBUFS))
    mpool = ctx.enter_context(tc.tile_pool(name="mp", bufs=2))
    ps_conv = ctx.enter_context(tc.tile_pool(name="psc", bufs=PS_BUFS, space="PSUM"))
    ps_z = ctx.enter_context(tc.tile_pool(name="psz", bufs=2, space="PSUM"))
    ps_s = ctx.enter_context(tc.tile_pool(name="pss", bufs=2, space="PSUM"))

    tiles = _make_tiles()

    def fold_unit(o, R, h):
        # Column-folded last row-tile: 4 width-segments of one half stacked
        # on 32-partition blocks, block-diagonal bands, ops span all 128
        # partitions (off-band lanes hold finite junk; masked exp gives
        # E=1 and the extended Z band keeps Z>0 there).
        mk = mpool.tile([128, 1], F32, tag="mk")
        nc.vector.memset(mk, 0.0)
        for b in range(4):
            nc.sync.dma_start(
                out=mk[32 * b : 32 * b + R + 4], in_=mask_d[o + 1 : o + R + 5, :]
            )
        if True:
            if True:
                g0 = h * WH
                X = xpool.tile([128, WH + 6], F32, tag="X")
                for b in range(4):
                    nc.sync.dma_start(
                        out=X[32 * b : 32 * b + 32, : SEGW + 6],
                        in_=xh_d[o : o + 32, g0 + b * SEGW : g0 + b * SEGW + SEGW + 6],
                    )
                Xhi = xhip.tile([128, WH + 6], F32R, tag="Xhi")
                nc.vector.tensor_copy(out=Xhi[:, : SEGW + 6], in_=X[:, : SEGW + 6])
                Xlo = xlop.tile([128, WH + 6], F32R, tag="Xlo")
                nc.vector.tensor_sub(
                    out=Xlo[:, : SEGW + 6],
                    in0=X[:, : SEGW + 6],
                    in1=Xhi[:, : SEGW + 6],
                )

                E = epool.tile([128, WH + 4], F32R, tag="E")
                for cs, cl in _chunks(SEGW + 4):
                    pc = ps_conv.tile([128, C], F32, tag="pc")
                    mms = []
                    for v in range(3):
                        mms.append((MhiF[v], Xhi, v))
                        mms.append((MloF[v], Xhi, v))
                    for v in range(3):
                        mms.append((MhiF[v], Xlo, v))
                    for i, (mband, xop, v) in enumerate(mms):
                        nc.tensor.matmul(
                            pc[:, :cl],
                            mband,
                            xop[:, cs + v : cs + v + cl],
                            start=(i == 0),
                            stop=(i == len(mms) - 1),
                        )
                    nc.scalar.activation(E[:, cs : cs + cl], pc[:, :cl], Exp, scale=mk)
                if h == 0:
                    nc.vector.memset(E[0:32, 0:2].bitcast(F32), 1.0)
                if h == WS - 1:
                    nc.vector.memset(E[96:128, SEGW + 2 : SEGW + 4].bitcast(F32), 1.0)

                U = upool.tile([128, WH + 2], F32R, tag="U")
                for cs, cl in _chunks(SEGW + 2):
                    pz = ps_z.tile([128, C], F32, tag="pz")
                    for v in range(3):
                        nc.tensor.matmul(
                            pz[:, :cl],
                            BTF,
                            E[:, cs + v : cs + v + cl],
                            start=(v == 0),
                            stop=(v == 2),
                        )
                    Rz = rzpool.tile([128, C], F32, tag="Rz")
                    nc.vector.reciprocal_approx_fast(out=Rz[:, :cl], in_=pz[:, :cl])
                    nc.vector.tensor_mul(
                        out=U[:, cs : cs + cl],
                        in0=X[:, cs + 2 : cs + 2 + cl],
                        in1=Rz[:, :cl],
                    )
                if h == 0:
                    nc.vector.memset(U[0:32, 0:1].bitcast(F32), 0.0)
                if h == WS - 1:
                    nc.vector.memset(U[96:128, SEGW + 1 : SEGW + 2].bitcast(F32), 0.0)

                res = respool.tile([128, WH], F32, tag="res")
                for cs, cl in _chunks(SEGW):
                    ps = ps_s.tile([128, C], F32, tag="ps")
                    for v in range(3):
                        nc.tensor.matmul(
                            ps[:, :cl],
                            BBF,
                            U[:, cs + v : cs + v + cl],
                            start=(v == 0),
                            stop=(v == 2),
                        )
                    nc.vector.tensor_mul(
                        out=res[:, cs : cs + cl],
                        in0=E[:, cs + 2 : cs + 2 + cl],
                        in1=ps[:, :cl],
                    )
                for b in range(4):
                    nc.sync.dma_start(
                        out=out_d[o : o + R, g0 + b * SEGW : g0 + (b + 1) * SEGW],
                        in_=res[32 * b + 2 : 32 * b + 2 + R, :SEGW],
                    )
            return

    def normal_tile(o, R):
        mk = mpool.tile([128, 1], F32, tag="mk")
        nc.sync.dma_start(out=mk[: R + 4], in_=mask_d[o + 1 : o + R + 5, :])
        for h in range(WS):
            g0 = h * WH
            # X[p, j] <-> (row r-3+p, global col g0-3+j)
            X = xpool.tile([128, WH + 6], F32, tag="X")
            nc.sync.dma_start(
                out=X[: R + 6, :], in_=xh_d[o : o + R + 6, g0 : g0 + WH + 6]
            )
            Xhi = xhip.tile([128, WH + 6], F32R, tag="Xhi")
            nc.vector.tensor_copy(out=Xhi[: R + 6, :], in_=X[: R + 6, :])
            Xlo = xlop.tile([128, WH + 6], F32R, tag="Xlo")
            nc.vector.tensor_sub(
                out=Xlo[: R + 6, :], in0=X[: R + 6, :], in1=Xhi[: R + 6, :]
            )

            # conv + exp -> E[m, e] <-> (row r-2+m, global col g0-2+e)
            E = epool.tile([128, WH + 4], F32R, tag="E")
            for cs, cl in _chunks(WH + 4):
                pc = ps_conv.tile([128, C], F32, tag="pc")
                mms = []
                for v in range(3):
                    mms.append((Mhi[v], Xhi, v))
                    mms.append((Mlo[v], Xhi, v))
                for v in range(3):
                    mms.append((Mhi[v], Xlo, v))
                for i, (mband, xop, v) in enumerate(mms):
                    nc.tensor.matmul(
                        pc[: R + 4, :cl],
                        mband[: R + 6, : R + 4],
                        xop[: R + 6, cs + v : cs + v + cl],
                        start=(i == 0),
                        stop=(i == len(mms) - 1),
                    )
                nc.scalar.activation(
                    E[: R + 4, cs : cs + cl],
                    pc[: R + 4, :cl],
                    Exp,
                    scale=mk[: R + 4],
                )
            # global-edge columns of E represent pad pixels: exp(0) = 1
            if h == 0:
                nc.vector.memset(E[: R + 4, 0:2].bitcast(F32), 1.0)
            if h == WS - 1:
                nc.vector.memset(E[: R + 4, WH + 2 : WH + 4].bitcast(F32), 1.0)

            # Z (vertical via BT, X frame) -> Rz -> U[m, z] (global col g0-1+z)
            U = upool.tile([128, WH + 2], F32R, tag="U")
            for cs, cl in _chunks(WH + 2):
                pz = ps_z.tile([128, C], F32, tag="pz")
                for v in range(3):
                    nc.tensor.matmul(
                        pz[: R + 4, :cl],
                        BT[: R + 4, : R + 4],
                        E[: R + 4, cs + v : cs + v + cl],
                        start=(v == 0),
                        stop=(v == 2),
                    )
                Rz = rzpool.tile([128, C], F32, tag="Rz")
                nc.vector.reciprocal_approx_fast(
                    out=Rz[: R + 4, :cl], in_=pz[: R + 4, :cl]
                )
                nc.vector.tensor_mul(
                    out=U[: R + 4, cs : cs + cl],
                    in0=X[: R + 4, cs + 2 : cs + 2 + cl],
                    in1=Rz[: R + 4, :cl],
                )
            # U at global-edge pad columns is 0 (fold drops OOB)
            if h == 0:
                nc.vector.memset(U[: R + 4, 0:1].bitcast(F32), 0.0)
            if h == WS - 1:
                nc.vector.memset(U[: R + 4, WH + 1 : WH + 2].bitcast(F32), 0.0)

            # S (vertical via BB, E frame) + res = E * S
            res = respool.tile([128, WH], F32, tag="res")
            for cs, cl in _chunks(WH):
                ps = ps_s.tile([128, C], F32, tag="ps")
                for v in range(3):
                    nc.tensor.matmul(
                        ps[: R + 2, :cl],
                        BB[: R + 4, : R + 2],
                        U[: R + 4, cs + v : cs + v + cl],
                        start=(v == 0),
                        stop=(v == 2),
                    )
                nc.vector.tensor_mul(
                    out=res[: R + 2, cs : cs + cl],
                    in0=E[: R + 2, cs + 2 : cs + 2 + cl],
                    in1=ps[: R + 2, :cl],
                )
            # valid output rows sit at partitions [2, R+2)
            nc.sync.dma_start(
                out=out_d[o : o + R, g0 : g0 + WH], in_=res[2 : R + 2, :WH]
            )

    of, Rf = tiles[-1]
    if len(tiles) > 1 and Rf <= 26:
        # cheap folded units at both pipeline edges: fast fill and drain
        fold_unit(of, Rf, 0)
        for o, R in tiles[:-1]:
            normal_tile(o, R)
        fold_unit(of, Rf, WS - 1)
    else:
        for o, R in tiles:
            normal_tile(o, R)


_CACHE: dict = {}


def _build():
    if "nc" in _CACHE:
        return _CACHE["nc"]
    nc = bacc.Bacc(
        "TRN2", target_bir_lowering=False, debug=False, num_devices=N_CORES
    )
    xh_d = nc.dram_tensor(
        "xh", (RC + 2 * HALO + 26, W + 2 * HALO), F32, kind="ExternalInput"
    ).ap()
    mask_d = nc.dram_tensor("mask", (RC + 2 * HALO, 1), F32, kind="ExternalInput").ap()
    bands_d = nc.dram_tensor("bands", (10, 128, 128), F32, kind="ExternalInput").ap()
    out_d = nc.dram_tensor("out", (RC, W), F32, kind="ExternalOutput").ap()
    with tile.TileContext(nc) as tc:
        _energy_body(tc, out_d, xh_d, mask_d, bands_d)
    nc.compile()
    _CACHE["nc"] = nc
    return nc


def kernel(shareable_energy: np.ndarray, kernel: np.ndarray, **_run_kw) -> np.ndarray:
    x = np.ascontiguousarray(np.asarray(shareable_energy, np.float32))
    k = np.asarray(kernel, np.float32)
    assert x.shape == (H, W), x.shape
    nc = _build()
    bands = _make_bands(k)
    in_maps = [_make_core_inputs(x, bands, core) for core in range(N_CORES)]
    r = run_bass_kernel_spmd(nc, in_maps, core_ids=list(range(N_CORES)), **_run_kw)
    out = np.concatenate([res["out"] for res in r.results], axis=0)
    if _run_kw:
        _CACHE["last_result"] = r
    return out
